# revision 29
# baseline (speedup 1.0000x reference)
"""Single-level 2D Haar DWT (periodization mode) on Trainium2.

Input x: (8, 512, 512, 16) fp32 NHWC. Output: (LL, LH, HL, HH), each
(8, 256, 256, 16) fp32 — +/- combinations of each 2x2 spatial block,
scaled by 0.5.

Sharding: pure data parallel — one batch sample per NeuronCore (8 cores).

The kernel is HBM/fabric-bandwidth bound (memory regime). All device
I/O is fp16: the host pre-scales x by 0.5 (exact) and downcasts to
fp16 (rel err ~8e-4, tolerance 2e-2), then upcasts the fp16 outputs.
Per-core traffic is 16.8 MB; the measured combined DMA ceiling is
~420 GB/s (SBUF AXI fabric), so the transfer floor is ~40us plus
~11us of fixed NEFF startup/teardown.

Work splits by W-columns across two compute paths (x viewed per core
as (512, 8192)):

Path A (cols 0:5120) — TensorE + ScalarE + VectorE per 128-row chunk:
  fp16 matmul H-butterfly -> ACT copy-downcast to fp16 SBUF -> DVE
  W-butterfly (fp16 tensor_tensor, 2x_1P mode). The 128x128 +/-1
  weight interleaves the bands: H-low of row pair r lands in PSUM row
  2r, H-high in row 2r+1. After the W-butterfly, one [128, 2560] tile
  holds (LL,HL) row-interleaved, so ONE plain-2D DMA (64 descriptors
  of 20KB) writes both subbands into a fused (256, 2*2560) DRAM
  tensor. Same for (LH,HH). Fused tensors alternate by chunk parity
  so no two in-flight DMAs target one DRAM tensor (Tile serializes
  same-tensor DMAs). Each chunk's input lands as 2048+3072-col tiles
  so group-0 matmuls start before the whole chunk arrives.

Path B (cols 5120:8192) — VectorE only: row pairs on partitions,
2-op H butterfly, then the W butterfly writes LL|LH (and HL|HH)
side-by-side into one [128, 3072] tile -> one 2D DMA each.

Schedule (tuned against perfetto traces):
  - B0's inputs lead the read stream: its outputs are ready ~5us
    after its data lands, so the write stream starts ~18us.
  - B1 second-to-last, A3 last: the final A chunk's matmul/ACT tail
    overlaps B1's DVE work.
  - 12 output DMAs total ride the Sync HWDGE ring (a second HWDGE
    ring measurably slows both); inputs ride the GpSimd SWDGE ring.
  - Host reassembles subbands from the fused tensors (cheap numpy).
"""

import sys

if "/opt/trn_rl_repo" not in sys.path:
    sys.path.insert(0, "/opt/trn_rl_repo")

import numpy as np

B, H, W, C = 8, 512, 512, 16
N_CORES = 8
HO, WO = H // 2, W // 2  # 256, 256
ROW = W * C  # 8192 elements per input row
OROW = WO * C  # 4096 elements per output row

A_W = 5120  # path A input columns
B_W = ROW - A_W  # 3072 path B input columns
A_OW = A_W // 2  # 2560 output columns from path A
B_OW = B_W // 2  # 1536 output columns from path B
GN = 2048  # max PSUM group (4 banks)
MM_N = 512  # one matmul / PSUM bank
A_GROUPS = (2048, 2048, 1024)

_CACHE = {}


def _haar_weight():
    """lhsT [k, m]: matmul computes out[m, n] = sum_k w[k, m] x[k, n].

    Band-interleaved: H-low of input row pair r -> PSUM row 2r,
    H-high -> PSUM row 2r+1 (so one output tile holds both subbands
    row-interleaved and a single 2D DMA can store them fused)."""
    w = np.zeros((128, 128), dtype=np.float16)
    for r in range(64):
        w[2 * r, 2 * r] = 1.0
        w[2 * r + 1, 2 * r] = 1.0
        w[2 * r, 2 * r + 1] = 1.0
        w[2 * r + 1, 2 * r + 1] = -1.0
    return w


def _build():
    import concourse.bacc as bacc
    import concourse.mybir as mybir
    import concourse.tile as tile

    fp16 = mybir.dt.float16
    fp32 = mybir.dt.float32

    nc = bacc.Bacc(
        "TRN2", target_bir_lowering=False, debug=False, num_devices=N_CORES
    )
    x = nc.dram_tensor("x", (H, ROW), fp16, kind="ExternalInput")
    wdram = nc.dram_tensor("w", (128, 128), fp16, kind="ExternalInput")
    # fused A outputs, one tensor per (pair, chunk): row r holds
    # [band0 | band1] of output row kc*64+r; "sum" pairs (LL, HL),
    # "diff" pairs (LH, HH). Per-chunk tensors mean no two DMAs ever
    # target one DRAM tensor (Tile serializes same-tensor DMAs).
    a_outs = {
        (pair, kc): nc.dram_tensor(f"{pair}_a{kc}", (64, 2 * A_OW), fp16,
                                   kind="ExternalOutput")
        for pair in ("sum", "diff")
        for kc in range(4)
    }
    # fused B outputs: row q holds [LL | LH] ("sumpair") resp.
    # [HL | HH] ("diffpair") of output row q
    b_outs = {
        name: nc.dram_tensor(f"{name}_b", (HO, 2 * B_OW), fp16,
                             kind="ExternalOutput")
        for name in ("sumpair", "diffpair")
    }

    xq = x.rearrange("(q t) m -> q t m", t=2)  # [pair, row-parity, cols]

    def emit_a_unit(nc, pools, wt, kc, split_input):
        """Path A chunk kc: input rows kc*128..+128, cols 0:A_W.

        split_input (first/last chunk): input lands as 2048+3072-col
        tiles so group-0 matmuls start before the whole chunk arrives.
        Middle chunks load in one DMA (fewer SWDGE descgen stalls)."""
        inpA, inpAm, psum, sbp, outA = pools
        rows = slice(kc * 128, (kc + 1) * 128)
        if split_input:
            xa = inpA.tile([128, GN], fp16, tag="xa")
            xb = inpA.tile([128, A_W - GN], fp16, tag="xb")
            nc.gpsimd.dma_start(xa[:], x[rows, 0:GN])
            nc.gpsimd.dma_start(xb[:], x[rows, GN:A_W])
            srcs = [(xa, 0), (xb, 0), (xb, GN)]
        else:
            xt = inpAm.tile([128, A_W], fp16, tag="xt")
            nc.gpsimd.dma_start(xt[:], x[rows, 0:A_W])
            srcs = [(xt, 0), (xt, GN), (xt, 2 * GN)]
        sb = sbp.tile([128, A_W], fp16)
        goff = 0
        for (src, soff), gsz in zip(srcs, A_GROUPS):
            ps = psum.tile([128, GN], fp32)
            for j in range(gsz // MM_N):
                lo = j * MM_N
                nc.tensor.matmul(
                    ps[:, lo : lo + MM_N],
                    wt[:],
                    src[:, soff + lo : soff + lo + MM_N],
                    start=True,
                    stop=True,
                )
            nc.scalar.copy(sb[:, goff : goff + gsz], ps[:, 0:gsz])
            goff += gsz
        sum_t = outA.tile([128, A_OW], fp16, tag="sum")
        diff_t = outA.tile([128, A_OW], fp16, tag="diff")
        sv_in = sb[:].rearrange("p (w u c) -> p w u c", u=2, c=C)
        ev, od = sv_in[:, :, 0, :], sv_in[:, :, 1, :]
        sv = sum_t[:].rearrange("p (w c) -> p w c", c=C)
        dv = diff_t[:].rearrange("p (w c) -> p w c", c=C)
        # one plain-2D DMA per TT: dst row r <- partitions 2r, 2r+1
        # (64 descriptors of 2*A_OW contiguous elements)
        nc.vector.tensor_add(sv, ev, od)
        nc.sync.dma_start(a_outs[("sum", kc)][:, :], sum_t[:])
        nc.vector.tensor_sub(dv, ev, od)
        nc.sync.dma_start(a_outs[("diff", kc)][:, :], diff_t[:])

    def emit_b_input(nc, inpB, pc):
        top = inpB.tile([128, B_W], fp16, tag="top")
        bot = inpB.tile([128, B_W], fp16, tag="bot")
        qs = slice(pc * 128, (pc + 1) * 128)
        ws = slice(A_W, ROW)
        nc.gpsimd.dma_start(top[:], xq[qs, 0, ws])
        nc.gpsimd.dma_start(bot[:], xq[qs, 1, ws])
        return top, bot

    def emit_b_unit(nc, pools, pc, top, bot):
        """Path B: 128 row-pairs pc*128..+128, input cols A_W:ROW."""
        midB, outB = pools
        qs = slice(pc * 128, (pc + 1) * 128)
        sum_b = midB.tile([128, B_W], fp16, tag="sum")
        diff_b = midB.tile([128, B_W], fp16, tag="diff")
        nc.vector.tensor_add(sum_b[:], top[:], bot[:])
        nc.vector.tensor_sub(diff_b[:], top[:], bot[:])
        WQ = B_W // (2 * C)  # 96 W-pairs
        for name, src in (("sumpair", sum_b), ("diffpair", diff_b)):
            s_in = src[:].rearrange("p (w u c) -> p w u c", u=2, c=C)
            ev, od = s_in[:, :, 0, :], s_in[:, :, 1, :]
            ot = outB.tile([128, 2 * B_OW], fp16, tag=name)
            ov = ot[:].rearrange("p (h w c) -> p h w c", h=2, c=C)
            nc.vector.tensor_add(ov[:, 0], ev, od)
            nc.vector.tensor_sub(ov[:, 1], ev, od)
            nc.sync.dma_start(b_outs[name][qs, :], ot[:])

    with tile.TileContext(nc) as tc:
        with (
            tc.tile_pool(name="wpool", bufs=1) as wpool,
            tc.tile_pool(name="inpA", bufs=2) as inpA,
            tc.tile_pool(name="inpAm", bufs=2) as inpAm,
            tc.tile_pool(name="psum", bufs=2, space="PSUM") as psum,
            tc.tile_pool(name="sbp", bufs=2) as sbp,
            tc.tile_pool(name="outA", bufs=2) as outA,
            tc.tile_pool(name="inpB", bufs=2) as inpB,
            tc.tile_pool(name="midB", bufs=2) as midB,
            tc.tile_pool(name="outB", bufs=2) as outB,
        ):
            wt = wpool.tile([128, 128], fp16)
            nc.sync.dma_start(wt[:], wdram[:])
            a_pools = (inpA, inpAm, psum, sbp, outA)
            b_pools = (midB, outB)
            # B0 first in both the read stream and the DVE queue; B1
            # second-to-last so the DVE is clear for A3's butterflies
            b0 = emit_b_input(nc, inpB, 0)
            emit_b_unit(nc, b_pools, 0, *b0)
            emit_a_unit(nc, a_pools, wt, 0, split_input=True)
            emit_a_unit(nc, a_pools, wt, 1, split_input=False)
            emit_a_unit(nc, a_pools, wt, 2, split_input=False)
            b1 = emit_b_input(nc, inpB, 1)
            emit_b_unit(nc, b_pools, 1, *b1)
            emit_a_unit(nc, a_pools, wt, 3, split_input=True)

    nc.compile()
    return nc


def _get_nc():
    if "nc" not in _CACHE:
        _CACHE["nc"] = _build()
    return _CACHE["nc"]


def _in_maps(x):
    w = _haar_weight()
    x16 = (x * np.float32(0.5)).astype(np.float16)
    return [
        {"x": np.ascontiguousarray(x16[i].reshape(H, ROW)), "w": w}
        for i in range(B)
    ]


def _assemble(res_i):
    """Reassemble (LL, LH, HL, HH) fp32 (HO, WO, C) for one core.

    A-path: fused tensor "sum_a{p}" row r = [LL row | HL row] of
    output row kc*64+r for the parity-p chunks; "diff" = (LH, HH).
    B-path: "sumpair_b" row q = [LL cols | LH cols]; "diffpair_b"
    row q = [HL cols | HH cols]."""
    a_band = {}
    for pair, (n0, n1) in (("sum", ("LL", "HL")), ("diff", ("LH", "HH"))):
        full = np.empty((HO, 2, A_OW), dtype=np.float16)
        for kc in range(4):
            rs = slice(kc * 64, (kc + 1) * 64)
            full[rs] = res_i[f"{pair}_a{kc}"].reshape(64, 2, A_OW)
        a_band[n0] = full[:, 0, :]
        a_band[n1] = full[:, 1, :]
    b_band = {}
    for pair, (n0, n1) in (
        ("sumpair", ("LL", "LH")),
        ("diffpair", ("HL", "HH")),
    ):
        t = res_i[f"{pair}_b"].reshape(HO, 2, B_OW)
        b_band[n0] = t[:, 0, :]
        b_band[n1] = t[:, 1, :]
    out = {}
    for name in ("LL", "LH", "HL", "HH"):
        w_full = np.concatenate([a_band[name], b_band[name]], axis=1)
        out[name] = w_full.astype(np.float32).reshape(HO, WO, C)
    return out


def kernel(x):
    from concourse.bass_utils import run_bass_kernel_spmd

    x = np.asarray(x, dtype=np.float32)
    assert x.shape == (B, H, W, C), x.shape

    nc = _get_nc()
    try:
        res = run_bass_kernel_spmd(nc, _in_maps(x), list(range(N_CORES)))
    except Exception:
        # transient NRT device errors have been observed right after
        # compile; one retry has always succeeded
        res = run_bass_kernel_spmd(nc, _in_maps(x), list(range(N_CORES)))

    per_core = [_assemble(res.results[i]) for i in range(B)]
    out = []
    for name in ("LL", "LH", "HL", "HH"):
        out.append(np.stack([pc[name] for pc in per_core], axis=0))
    return tuple(out)


# revision 34
# speedup vs baseline: 1.1016x; 1.1016x over previous
"""Single-level 2D Haar DWT (periodization mode) on Trainium2.

Input x: (8, 512, 512, 16) fp32 NHWC. Output: (LL, LH, HL, HH), each
(8, 256, 256, 16) fp32 — +/- combinations of each 2x2 spatial block,
scaled by 0.5.

Sharding: pure data parallel — one batch sample per NeuronCore (8 cores).

The kernel is HBM/fabric-bandwidth bound (memory regime). All device
I/O is fp16: the host pre-scales x by 0.5 (exact) and downcasts to
fp16 (rel err ~8e-4, tolerance 2e-2), then upcasts the fp16 outputs.
Per-core traffic is 16.8 MB; the measured combined DMA ceiling is
~420 GB/s (SBUF AXI fabric), so the transfer floor is ~40us plus
~11us of fixed NEFF startup/teardown.

Work splits by W-columns across two compute paths (x viewed per core
as (512, 8192)):

Path A (cols 0:5120) — TensorE + ScalarE + VectorE per 128-row chunk:
  fp16 matmul H-butterfly -> ACT copy-downcast to fp16 SBUF -> DVE
  W-butterfly (fp16 tensor_tensor, 2x_1P mode). The 128x128 +/-1
  weight interleaves the bands: H-low of row pair r lands in PSUM row
  2r, H-high in row 2r+1. After the W-butterfly, one [128, 2560] tile
  holds (LL,HL) row-interleaved, so ONE plain-2D DMA (64 descriptors
  of 20KB) writes both subbands into a fused (256, 2*2560) DRAM
  tensor. Same for (LH,HH). Fused tensors alternate by chunk parity
  so no two in-flight DMAs target one DRAM tensor (Tile serializes
  same-tensor DMAs). Each chunk's input lands as 2048+3072-col tiles
  so group-0 matmuls start before the whole chunk arrives.

Path B (cols 5120:8192) — VectorE only: row pairs on partitions,
2-op H butterfly, then the W butterfly writes LL|LH (and HL|HH)
side-by-side into one [128, 3072] tile -> one 2D DMA each.

Schedule (tuned against perfetto traces):
  - B0's inputs lead the read stream: its outputs are ready ~5us
    after its data lands, so the write stream starts ~18us.
  - B1 second-to-last, A3 last: the final A chunk's matmul/ACT tail
    overlaps B1's DVE work.
  - 12 output DMAs total ride the Sync HWDGE ring (a second HWDGE
    ring measurably slows both); inputs ride the GpSimd SWDGE ring.
  - Host reassembles subbands from the fused tensors (cheap numpy).
"""

import sys

if "/opt/trn_rl_repo" not in sys.path:
    sys.path.insert(0, "/opt/trn_rl_repo")

import numpy as np

B, H, W, C = 8, 512, 512, 16
N_CORES = 8
HO, WO = H // 2, W // 2  # 256, 256
ROW = W * C  # 8192 elements per input row
OROW = WO * C  # 4096 elements per output row

A_W = 5120  # path A input columns
B_W = ROW - A_W  # 3072 path B input columns
A_OW = A_W // 2  # 2560 output columns from path A
B_OW = B_W // 2  # 1536 output columns from path B
GN = 2048  # max PSUM group (4 banks)
MM_N = 512  # one matmul / PSUM bank
A_GROUPS = (2048, 2048, 1024)

_CACHE = {}


def _haar_weight():
    """lhsT [k, m]: matmul computes out[m, n] = sum_k w[k, m] x[k, n].

    Band-interleaved: H-low of input row pair r -> PSUM row 2r,
    H-high -> PSUM row 2r+1 (so one output tile holds both subbands
    row-interleaved and a single 2D DMA can store them fused)."""
    w = np.zeros((128, 128), dtype=np.float16)
    for r in range(64):
        w[2 * r, 2 * r] = 1.0
        w[2 * r + 1, 2 * r] = 1.0
        w[2 * r, 2 * r + 1] = 1.0
        w[2 * r + 1, 2 * r + 1] = -1.0
    return w


def _build():
    import concourse.bacc as bacc
    import concourse.mybir as mybir
    import concourse.tile as tile

    fp16 = mybir.dt.float16
    fp32 = mybir.dt.float32

    nc = bacc.Bacc(
        "TRN2", target_bir_lowering=False, debug=False, num_devices=N_CORES
    )
    x = nc.dram_tensor("x", (H, ROW), fp16, kind="ExternalInput")
    wdram = nc.dram_tensor("w", (128, 128), fp16, kind="ExternalInput")
    # fused A outputs, one tensor per (pair, chunk): row r holds
    # [band0 | band1] of output row kc*64+r; "sum" pairs (LL, HL),
    # "diff" pairs (LH, HH). Per-chunk tensors mean no two DMAs ever
    # target one DRAM tensor (Tile serializes same-tensor DMAs).
    a_outs = {
        (pair, kc): nc.dram_tensor(f"{pair}_a{kc}", (64, 2 * A_OW), fp16,
                                   kind="ExternalOutput")
        for pair in ("sum", "diff")
        for kc in range(4)
    }
    # fused B outputs: row q holds [LL | LH] ("sumpair") resp.
    # [HL | HH] ("diffpair") of output row q
    b_outs = {
        name: nc.dram_tensor(f"{name}_b", (HO, 2 * B_OW), fp16,
                             kind="ExternalOutput")
        for name in ("sumpair", "diffpair")
    }

    xq = x.rearrange("(q t) m -> q t m", t=2)  # [pair, row-parity, cols]

    def emit_a_unit(nc, pools, wt, kc):
        """Path A chunk kc: input rows kc*128..+128, cols 0:A_W."""
        inpA, psum, sbp, outA = pools
        rows = slice(kc * 128, (kc + 1) * 128)
        xa = inpA.tile([128, GN], fp16, tag="xa")
        xb = inpA.tile([128, A_W - GN], fp16, tag="xb")
        nc.gpsimd.dma_start(xa[:], x[rows, 0:GN])
        nc.gpsimd.dma_start(xb[:], x[rows, GN:A_W])
        srcs = [(xa, 0), (xb, 0), (xb, GN)]
        sb = sbp.tile([128, A_W], fp16)
        goff = 0
        for (src, soff), gsz in zip(srcs, A_GROUPS):
            ps = psum.tile([128, GN], fp32)
            for j in range(gsz // MM_N):
                lo = j * MM_N
                nc.tensor.matmul(
                    ps[:, lo : lo + MM_N],
                    wt[:],
                    src[:, soff + lo : soff + lo + MM_N],
                    start=True,
                    stop=True,
                )
            nc.scalar.copy(sb[:, goff : goff + gsz], ps[:, 0:gsz])
            goff += gsz
        sum_t = outA.tile([128, A_OW], fp16, tag="sum")
        diff_t = outA.tile([128, A_OW], fp16, tag="diff")
        sv_in = sb[:].rearrange("p (w u c) -> p w u c", u=2, c=C)
        ev, od = sv_in[:, :, 0, :], sv_in[:, :, 1, :]
        sv = sum_t[:].rearrange("p (w c) -> p w c", c=C)
        dv = diff_t[:].rearrange("p (w c) -> p w c", c=C)
        # one plain-2D DMA per TT: dst row r <- partitions 2r, 2r+1
        # (64 descriptors of 2*A_OW contiguous elements)
        nc.vector.tensor_add(sv, ev, od)
        nc.sync.dma_start(a_outs[("sum", kc)][:, :], sum_t[:])
        nc.vector.tensor_sub(dv, ev, od)
        nc.sync.dma_start(a_outs[("diff", kc)][:, :], diff_t[:])

    def emit_b_input(nc, inpB, pc, eng):
        top = inpB.tile([128, B_W], fp16, tag="top")
        bot = inpB.tile([128, B_W], fp16, tag="bot")
        qs = slice(pc * 128, (pc + 1) * 128)
        ws = slice(A_W, ROW)
        eng.dma_start(top[:], xq[qs, 0, ws])
        eng.dma_start(bot[:], xq[qs, 1, ws])
        return top, bot

    def emit_b_unit(nc, pools, pc, top, bot):
        """Path B: 128 row-pairs pc*128..+128, input cols A_W:ROW."""
        midB, outB = pools
        qs = slice(pc * 128, (pc + 1) * 128)
        sum_b = midB.tile([128, B_W], fp16, tag="sum")
        diff_b = midB.tile([128, B_W], fp16, tag="diff")
        nc.vector.tensor_add(sum_b[:], top[:], bot[:])
        nc.vector.tensor_sub(diff_b[:], top[:], bot[:])
        WQ = B_W // (2 * C)  # 96 W-pairs
        for name, src in (("sumpair", sum_b), ("diffpair", diff_b)):
            s_in = src[:].rearrange("p (w u c) -> p w u c", u=2, c=C)
            ev, od = s_in[:, :, 0, :], s_in[:, :, 1, :]
            ot = outB.tile([128, 2 * B_OW], fp16, tag=name)
            ov = ot[:].rearrange("p (h w c) -> p h w c", h=2, c=C)
            nc.vector.tensor_add(ov[:, 0], ev, od)
            nc.vector.tensor_sub(ov[:, 1], ev, od)
            nc.sync.dma_start(b_outs[name][qs, :], ot[:])

    with tile.TileContext(nc) as tc:
        with (
            tc.tile_pool(name="wpool", bufs=1) as wpool,
            tc.tile_pool(name="inpA", bufs=4) as inpA,
            tc.tile_pool(name="psum", bufs=2, space="PSUM") as psum,
            tc.tile_pool(name="sbp", bufs=2) as sbp,
            tc.tile_pool(name="outA", bufs=2) as outA,
            tc.tile_pool(name="inpB", bufs=2) as inpB,
            tc.tile_pool(name="midB", bufs=2) as midB,
            tc.tile_pool(name="outB", bufs=2) as outB,
        ):
            wt = wpool.tile([128, 128], fp16)
            a_pools = (inpA, psum, sbp, outA)
            b_pools = (midB, outB)
            # B0's inputs lead on the Sync HWDGE ring: it boots ~1.5us
            # before the SWDGE path and is otherwise idle until the
            # first outputs (~18us), so the read stream starts earlier
            # and two descgens come off the GpSimd queue. B1 second-to-
            # last so the DVE is clear for A3's butterflies.
            b0 = emit_b_input(nc, inpB, 0, nc.sync)
            nc.sync.dma_start(wt[:], wdram[:])
            emit_b_unit(nc, b_pools, 0, *b0)
            emit_a_unit(nc, a_pools, wt, 0)
            emit_a_unit(nc, a_pools, wt, 1)
            emit_a_unit(nc, a_pools, wt, 2)
            b1 = emit_b_input(nc, inpB, 1, nc.gpsimd)
            emit_b_unit(nc, b_pools, 1, *b1)
            emit_a_unit(nc, a_pools, wt, 3)

    nc.compile()
    return nc


def _get_nc():
    if "nc" not in _CACHE:
        _CACHE["nc"] = _build()
    return _CACHE["nc"]


def _in_maps(x):
    w = _haar_weight()
    x16 = (x * np.float32(0.5)).astype(np.float16)
    return [
        {"x": np.ascontiguousarray(x16[i].reshape(H, ROW)), "w": w}
        for i in range(B)
    ]


def _assemble(res_i):
    """Reassemble (LL, LH, HL, HH) fp32 (HO, WO, C) for one core.

    A-path: fused tensor "sum_a{p}" row r = [LL row | HL row] of
    output row kc*64+r for the parity-p chunks; "diff" = (LH, HH).
    B-path: "sumpair_b" row q = [LL cols | LH cols]; "diffpair_b"
    row q = [HL cols | HH cols]."""
    a_band = {}
    for pair, (n0, n1) in (("sum", ("LL", "HL")), ("diff", ("LH", "HH"))):
        full = np.empty((HO, 2, A_OW), dtype=np.float16)
        for kc in range(4):
            rs = slice(kc * 64, (kc + 1) * 64)
            full[rs] = res_i[f"{pair}_a{kc}"].reshape(64, 2, A_OW)
        a_band[n0] = full[:, 0, :]
        a_band[n1] = full[:, 1, :]
    b_band = {}
    for pair, (n0, n1) in (
        ("sumpair", ("LL", "LH")),
        ("diffpair", ("HL", "HH")),
    ):
        t = res_i[f"{pair}_b"].reshape(HO, 2, B_OW)
        b_band[n0] = t[:, 0, :]
        b_band[n1] = t[:, 1, :]
    out = {}
    for name in ("LL", "LH", "HL", "HH"):
        w_full = np.concatenate([a_band[name], b_band[name]], axis=1)
        out[name] = w_full.astype(np.float32).reshape(HO, WO, C)
    return out


def kernel(x):
    from concourse.bass_utils import run_bass_kernel_spmd

    x = np.asarray(x, dtype=np.float32)
    assert x.shape == (B, H, W, C), x.shape

    nc = _get_nc()
    in_maps = _in_maps(x)
    out = None
    for attempt in range(3):
        # transient NRT device errors (and, rarely, NaN-poisoned output
        # from a degrading device) have been observed; finite inputs can
        # never produce non-finite Haar coefficients, so retry on either
        try:
            res = run_bass_kernel_spmd(nc, in_maps, list(range(N_CORES)))
        except Exception:
            if attempt == 2:
                raise
            continue
        per_core = [_assemble(res.results[i]) for i in range(B)]
        out = []
        for name in ("LL", "LH", "HL", "HH"):
            out.append(np.stack([pc[name] for pc in per_core], axis=0))
        if all(np.isfinite(o).all() for o in out):
            break
    return tuple(out)
